# revision 13
# baseline (speedup 1.0000x reference)
"""Trainium2 Bass kernel for nn_CrossSeqTransformer (S=4, B=1, C=128, h=w=d=10).

Strategy (8 NeuronCores, sequence-parallel over L = S*N = 4000 tokens):
  - Transposed token space: tokensT [C=128 partitions, L free]; each core owns
    a 500-token block of the L axis (queries/output), K/V replicated.
  - log_R bias reduces to a per-key column bias beta*log r_j fused into the
    softmax exp (row term cancels in softmax); no max-subtraction needed.
  - Softmax exp is split across TWO engines: the ACT engine computes real
    exp with fp8e4 output, and the Vector engine computes a Schraudolph-style
    exp (affine op + round-to-nearest saturating cast to uint8 == e4m3 bit
    pattern).  Both feed fp8 P tiles.
  - PV uses fp8 DoubleRow pair matmuls: stationary [Vh|0],[0|Vh1] slots make
    one instruction compute TWO heads' PV for a j-tile (PSUM dst partition 0
    is the only one hardware allows for DoubleRow).  Denominators come free
    via constant-one columns in the V stationary.
  - Layer-1 -> layer-2 AllGather ships fp8 tokens (layer-2 K/V weights are
    host-prescaled by 16 to keep fp8 operands out of subnormals; the 16x is
    folded back in the output projection and exp scale).
  - Scores stay bf16 (Dh=16 contraction gains nothing from fp8 pairs).
"""

import numpy as np
import ml_dtypes

S, B, C, N = 4, 1, 128, 1000
L = S * N                    # 4000
H, DH = 8, 16
NL = 2
NCORE = 8
BLK = L // NCORE             # 500
BLKP = 512                   # padded block
LP = 4096                    # padded L
NJT = LP // 128              # 32 j-tiles
SCALE = DH ** -0.5           # 0.25
COL = 500                    # live columns of the 512-wide block
LN_EPS = 1e-5
LOG2E = 1.4426950408889634
SIGMA = 0.0430               # Schraudolph shift
KV8 = 16.0                   # layer-2 fp8 K/V weight prescale
ACT_PAD = -30.0              # pad-token bias for ACT exp path
DVE_PAD = -4000.0            # pad bias for DVE bit-hack path (saturates to 0)
# jt tiles handled by the DVE exp path (rest go to ACT)
DVE_JTS = frozenset(jt for jt in range(NJT) if jt % 8 in (1, 3, 6))

_CACHE = {}


def _pin_act_tables():
    """Keep Exp/Ln resolvable only via natural_log_exp_and_others so the
    act-table-load pass doesn't thrash between exp/ln/gelu sets."""
    import concourse.bacc as bacc
    import concourse.mybir as mybir
    if getattr(bacc, "_act_tables_pinned", False):
        return
    orig = bacc.get_activation_tables

    def patched(arch):
        t = orig(arch)
        exp_f = mybir.ActivationFunctionType.Exp
        ln_f = mybir.ActivationFunctionType.Ln
        out = {}
        for name, funcs in t.items():
            if name != "natural_log_exp_and_others":
                funcs = funcs - {exp_f, ln_f}
            out[name] = funcs
        return out

    bacc.get_activation_tables = patched
    bacc._act_tables_pinned = True


def _build_nc():
    import concourse.bacc as bacc
    import concourse.mybir as mybir
    import concourse.tile as tile

    _pin_act_tables()

    F32 = mybir.dt.float32
    BF16 = mybir.dt.bfloat16
    FP8 = mybir.dt.float8e4
    U8 = mybir.dt.uint8
    AF = mybir.ActivationFunctionType
    OP = mybir.AluOpType
    PM = mybir.MatmulPerfMode.DoubleRow

    nc = bacc.Bacc("TRN2", target_bir_lowering=False, debug=False,
                   num_devices=NCORE)

    # ---- DRAM I/O ----------------------------------------------------------
    d_z = nc.dram_tensor("z3", [S, C, N], BF16, kind="ExternalInput").ap()
    d_zblk = nc.dram_tensor("zblk", [C, BLKP], F32, kind="ExternalInput").ap()
    d_seqe = nc.dram_tensor("seqe", [C, 1], F32, kind="ExternalInput").ap()
    d_seqeT = nc.dram_tensor("seqeT", [C, S], F32, kind="ExternalInput").ap()
    d_wk1 = nc.dram_tensor("wk1", [2, C, C], BF16, kind="ExternalInput").ap()
    d_wk2 = nc.dram_tensor("wk2", [2, C, C], FP8, kind="ExternalInput").ap()
    d_wq = nc.dram_tensor("wqp", [NL, 2, C, C], BF16, kind="ExternalInput").ap()
    d_wv1 = nc.dram_tensor("wv1", [C, 256], BF16, kind="ExternalInput").ap()
    d_wv2 = nc.dram_tensor("wv2", [C, 256], FP8, kind="ExternalInput").ap()
    d_wpp = nc.dram_tensor("wpp", [NL, 4, 64, C], BF16, kind="ExternalInput").ap()
    d_logrb = nc.dram_tensor("logrb", [C, NL * NJT], F32, kind="ExternalInput").ap()
    d_bvecb = nc.dram_tensor("bvecb", [C, NL * NJT], F32, kind="ExternalInput").ap()
    d_w1 = nc.dram_tensor("w1", [C, 4 * C], BF16, kind="ExternalInput").ap()
    d_w2 = nc.dram_tensor("w2", [4 * C, C], BF16, kind="ExternalInput").ap()
    d_b1t = nc.dram_tensor("b1t", [C, 4], F32, kind="ExternalInput").ap()
    d_b2 = nc.dram_tensor("b2c", [C, 1], F32, kind="ExternalInput").ap()
    d_ln1gn = nc.dram_tensor("ln1gn", [C, NL], F32, kind="ExternalInput").ap()
    d_ln1b = nc.dram_tensor("ln1b", [C, NL], F32, kind="ExternalInput").ap()
    d_ln2gn = nc.dram_tensor("ln2gn", [C, 1], F32, kind="ExternalInput").ap()
    d_ln2b = nc.dram_tensor("ln2b", [C, 1], F32, kind="ExternalInput").ap()
    d_out = nc.dram_tensor("outT", [C, BLK], F32, kind="ExternalOutput").ap()

    # E': denominator-broadcast matrix for head-pair PV tiles:
    # out row m (0..63) reads denominator row 16 (m<32) or 48 (m>=32)
    E = np.zeros((128, 128), np.float32)
    for m in range(64):
        E[16 + 32 * (m // 32), m] = 1.0
    d_emat = nc.inline_tensor(E.astype(ml_dtypes.bfloat16), name="ematp")
    d_ones = nc.inline_tensor(np.full((128, 128), 1.0 / 128, np.float32),
                              name="ones128")

    with tile.TileContext(nc, num_cores=NCORE, pool_alloc_mode="queue") as tc:
        with tc.tile_pool(name="sb", bufs=1) as sb, \
             tc.tile_pool(name="sx", bufs=3) as sx, \
             tc.tile_pool(name="se", bufs=6) as se, \
             tc.tile_pool(name="pso", bufs=2, space="PSUM") as pso, \
             tc.tile_pool(name="pss", bufs=3, space="PSUM") as pss, \
             tc.tile_pool(name="dram", bufs=1, space="DRAM") as dram:

            # ---- resident tiles -------------------------------------------
            tokT = sb.tile([128, LP], BF16)       # layer-1 tokens^T
            tokT2 = sb.tile([128, LP], FP8)       # layer-2 tokens^T (fp8)
            ktA = sb.tile([128, LP], BF16)        # K^T heads 0-3 (32-padded)
            ktB = sb.tile([128, LP], BF16)        # K^T heads 4-7
            qtA = sb.tile([128, BLKP], BF16)
            qtB = sb.tile([128, BLKP], BF16)
            vp8 = sb.tile([128, NJT * 512], FP8)  # V pair-slot blocks, fp8
            xblk = sb.tile([128, BLKP], F32)      # residual stream (block)
            xblkb = sb.tile([128, BLKP], BF16)
            xblk8 = sb.tile([128, BLKP], FP8)

            wk1s = sb.tile([128, 2 * 128], BF16)
            wk2s = sb.tile([128, 2 * 128], FP8)
            wqs = sb.tile([128, NL * 2 * 128], BF16)
            wv1s = sb.tile([128, 256], BF16)
            wv2s = sb.tile([128, 256], FP8)
            wpps = sb.tile([128, NL * 4 * 128], BF16)
            w1s = sb.tile([128, 512], BF16)
            w2s = sb.tile([128, 512], BF16)
            ematp = sb.tile([128, 128], BF16)
            onesc = sb.tile([128, 128], F32)
            zeroc = sb.tile([128, 1], F32)
            logrb = sb.tile([128, NL * NJT], F32)
            bvecb = sb.tile([128, NL * NJT], F32)
            b1t = sb.tile([128, 4], F32)
            b2c = sb.tile([128, 1], F32)
            ln1gn = sb.tile([128, NL], F32)
            ln1b = sb.tile([128, NL], F32)
            ln2gn = sb.tile([128, 1], F32)
            ln2b = sb.tile([128, 1], F32)
            seqeT = sb.tile([128, S], F32)
            seqe = sb.tile([128, 1], F32)
            epsc = sb.tile([128, 1], F32)
            g4 = sb.tile([128, 2048], BF16)       # gelu(hdn^T) chunks

            # ---- input + weight loads (tokenize path first: it gates K) ---
            nc.sync.dma_start(seqeT[:], d_seqeT)
            nc.sync.dma_start(seqe[:], d_seqe)
            nc.sync.dma_start(
                wk1s[:].rearrange("p (t b) -> p t b", t=2),
                d_wk1.rearrange("t a b -> a t b"))
            for s in range(S):
                stage = sx.tile([128, N], BF16, tag="stage")
                (nc.sync if s % 2 == 0 else nc.scalar).dma_start(stage[:], d_z[s])
                nc.vector.tensor_scalar_add(tokT[:, s * N:(s + 1) * N], stage[:],
                                            seqeT[:, s:s + 1])
            nc.gpsimd.memset(tokT[:, L:LP], 0.0)
            zstage = sx.tile([128, BLKP], F32, tag="stage")
            nc.sync.dma_start(zstage[:], d_zblk)
            nc.vector.tensor_scalar_add(xblk[:], zstage[:], seqe[:])
            nc.vector.tensor_copy(xblkb[:], xblk[:])

            nc.scalar.dma_start(wv1s[:], d_wv1)
            nc.scalar.dma_start(
                wqs[:].rearrange("p (l t b) -> p l t b", l=NL, t=2),
                d_wq.rearrange("l t a b -> a l t b"))
            nc.gpsimd.dma_start(
                wk2s[:].rearrange("p (t b) -> p t b", t=2),
                d_wk2.rearrange("t a b -> a t b"))
            nc.gpsimd.dma_start(wv2s[:], d_wv2)
            nc.gpsimd.dma_start(
                wpps[0:64, :].rearrange("p (l g b) -> p l g b", l=NL, g=4),
                d_wpp.rearrange("l g a b -> a l g b"))
            nc.gpsimd.dma_start(w1s[:], d_w1)
            nc.gpsimd.dma_start(
                w2s[:].rearrange("p (f c) -> p f c", f=4),
                d_w2.rearrange("(f p) c -> p f c", p=128))
            nc.gpsimd.dma_start(ematp[:], d_emat.ap())
            nc.gpsimd.dma_start(onesc[:], d_ones.ap())
            nc.vector.memset(zeroc[:], 0.0)
            nc.gpsimd.dma_start(logrb[:], d_logrb)
            nc.gpsimd.dma_start(bvecb[:], d_bvecb)
            nc.gpsimd.dma_start(b1t[:], d_b1t)
            nc.gpsimd.dma_start(b2c[:], d_b2)
            nc.scalar.dma_start(ln1gn[:], d_ln1gn)
            nc.scalar.dma_start(ln1b[:], d_ln1b)
            nc.scalar.dma_start(ln2gn[:], d_ln2gn)
            nc.scalar.dma_start(ln2b[:], d_ln2b)
            nc.vector.memset(epsc[:], LN_EPS)

            def wq_l(l, t):
                return wqs[:, (l * 2 + t) * 128:(l * 2 + t + 1) * 128]

            def wpp_g(l, g):
                return wpps[:, (l * 4 + g) * 128:(l * 4 + g + 1) * 128]

            def layernorm_cols(y, out_f32, out_bf16, gneg, bvec, c0, c1,
                               out_fp8=None):
                w = c1 - c0
                psL = pso.tile([128, BLKP], F32, tag="acc", name="psL")
                psL2 = pso.tile([128, BLKP], F32, tag="acc", name="psL2")
                nc.tensor.matmul(psL[:, 0:w], onesc[:], y[:, c0:c1],
                                 start=True, stop=True)       # mu (broadcast)
                t = sx.tile([128, BLKP], F32, tag="ln1", name="t")
                nc.vector.tensor_sub(t[:, 0:w], psL[:, 0:w], y[:, c0:c1])
                sq = sx.tile([128, BLKP], F32, tag="ln2", name="sq")
                nc.vector.tensor_mul(sq[:, 0:w], t[:, 0:w], t[:, 0:w])
                nc.tensor.matmul(psL2[:, 0:w], onesc[:], sq[:, 0:w],
                                 start=True, stop=True)       # var (broadcast)
                lnv = sx.tile([128, BLKP], F32, tag="ln0", name="lnv")
                nc.scalar.activation(lnv[:, 0:w], psL2[:, 0:w],
                                     AF.Ln, bias=epsc[:], scale=1.0)
                rstd = sx.tile([128, BLKP], F32, tag="ln2", name="rstd")
                nc.scalar.activation(rstd[:, 0:w], lnv[:, 0:w],
                                     AF.Exp, bias=zeroc[:], scale=-0.5)
                ts = sx.tile([128, BLKP], F32, tag="ln0", name="ts")
                nc.vector.tensor_mul(ts[:, 0:w], t[:, 0:w], rstd[:, 0:w])
                nc.vector.tensor_scalar(out=out_f32[:, c0:c1],
                                        in0=ts[:, 0:w],
                                        scalar1=gneg, scalar2=bvec,
                                        op0=OP.mult, op1=OP.add)
                nc.vector.tensor_copy(out_bf16[:, c0:c1], out_f32[:, c0:c1])
                if out_fp8 is not None:
                    nc.vector.tensor_copy(out_fp8[:, c0:c1], out_f32[:, c0:c1])

            def layernorm(y, out_f32, out_bf16, gneg, bvec, out_fp8=None):
                for (c0, c1) in ((0, 256), (256, COL)):
                    layernorm_cols(y, out_f32, out_bf16, gneg, bvec, c0, c1,
                                   out_fp8)

            vp8v = vp8[:].rearrange("p (j g s c) -> p j g s c",
                                    j=NJT, g=4, s=2)
            GROUPS = [(0, 1), (2, 3), (4, 5), (6, 7)]

            # vp8 zero/ones structure is layer-invariant: zero-fill and set
            # denominator ones columns ONCE (V copies only touch dim cols);
            # chunked so the memset doesn't monopolize gpsimd/SBUF for ~14us
            for ch in range(8):
                nc.gpsimd.memset(vp8[:, 2048 * ch:2048 * (ch + 1)], 0.0)
            for g in range(4):
                nc.gpsimd.memset(vp8v[:, :, g, 0, 16:17], 1.0)
                nc.gpsimd.memset(vp8v[:, :, g, 1, 48:49], 1.0)

            for l in range(NL):
                tok = tokT if l == 0 else tokT2
                wk_t = (lambda t: wk1s[:, t * 128:(t + 1) * 128]) if l == 0 \
                    else (lambda t: wk2s[:, t * 128:(t + 1) * 128])
                wv_s = wv1s if l == 0 else wv2s

                # ---- Q projection -----------------------------------------
                for t, qt in ((0, qtA), (1, qtB)):
                    psQ = pss.tile([128, 1024], F32, tag="s")
                    nc.tensor.matmul(psQ[:, 0:COL], wq_l(l, t),
                                     xblkb[:, 0:COL], start=True, stop=True)
                    nc.vector.tensor_copy(qt[:, 0:COL], psQ[:, 0:COL])

                # ---- K + V projections, interleaved by c4 chunk -----------
                for c4 in range(4):
                    for t, kt in ((0, ktA), (1, ktB)):
                        psK = pss.tile([128, 1024], F32, tag="s")
                        for hw_ in range(2):
                            nc.tensor.matmul(
                                psK[:, 512 * hw_:512 * (hw_ + 1)], wk_t(t),
                                tok[:, 1024 * c4 + 512 * hw_:
                                     1024 * c4 + 512 * (hw_ + 1)],
                                start=True, stop=True)
                        nc.vector.tensor_copy(
                            kt[:, 1024 * c4:1024 * (c4 + 1)], psK[:])
                    for jt in range(8 * c4, 8 * c4 + 8):
                        psV = pss.tile([128, 1024], F32, tag="s")
                        nc.tensor.matmul(psV[:, 0:256],
                                         tok[:, 128 * jt:128 * (jt + 1)],
                                         wv_s[:], start=True, stop=True)
                        psVg = psV[:, 0:256].rearrange(
                            "p (g s c) -> p g s c", g=4, s=2)
                        # even heads -> slot A cols 0:16; odd -> slot B 32:48
                        nc.vector.tensor_copy(
                            vp8v[:, jt, :, 0, 0:16], psVg[:, :, 0, 0:16])
                        nc.scalar.copy(
                            vp8v[:, jt, :, 1, 32:48], psVg[:, :, 1, 0:16])

                # ---- attention: two head-pair groups interleaved ----------
                # (4 distinct PE row positions in flight + 2 exp engines)
                stageN = {}

                def attn_scores(g, jt):
                    h0, h1 = GROUPS[g]
                    psS = pss.tile([128, 1024], F32, tag="s")
                    for i, hh in enumerate((h0, h1)):
                        kt, qt, base = ((ktA, qtA, 32 * hh) if hh < 4
                                        else (ktB, qtB, 32 * (hh - 4)))
                        nc.tensor.matmul(
                            psS[:, 512 * i:512 * i + COL],
                            kt[base:base + 16, 128 * jt:128 * (jt + 1)],
                            qt[base:base + 16, 0:COL],
                            start=True, stop=True, tile_position=(base, 0))
                    return psS

                def attn_exp_pv(g, jt, psS):
                    e8 = se.tile([128, 1024], FP8, tag="e8")
                    col = l * NJT + jt
                    e8v = e8[:].rearrange("p (t n) -> p t n", t=2)
                    e8u = e8[:].bitcast(U8)
                    sc = SCALE * (1.0 / KV8 if l == 1 else 1.0)
                    # exp split over ACT (true exp) and DVE (Schraudolph
                    # bit-hack); ACT takes ~62.5% of halves for balance
                    act_both = (jt % 4) == 3
                    dve_half = (jt + g) % 2
                    for half in range(2):
                        if act_both or half != dve_half:
                            nc.scalar.activation(
                                e8[:, 512 * half:512 * half + COL],
                                psS[:, 512 * half:512 * half + COL],
                                AF.Exp, bias=logrb[:, col:col + 1],
                                scale=sc)
                        else:
                            nc.vector.tensor_scalar(
                                out=e8u[:, 512 * half:512 * half + COL],
                                in0=psS[:, 512 * half:512 * half + COL],
                                scalar1=8.0 * LOG2E * sc,
                                scalar2=bvecb[:, col:col + 1],
                                op0=OP.mult, op1=OP.add)
                    nc.tensor.matmul(
                        psO[g][0:64, 0:COL], vp8v[:, jt, g],
                        e8v[:, :, 0:COL],
                        start=(jt == 0), stop=(jt == NJT - 1),
                        perf_mode=PM)

                def stage_n(g):
                    # normalize: att = num * exp(-ln denom); E' broadcasts
                    # the denominator rows 16/48 over the pair block
                    psOg = psO[g]
                    t8 = sx.tile([128, BLKP], BF16, tag=f"sn{g}")
                    nc.vector.tensor_copy(t8[0:64, 0:COL], psOg[0:64, 0:COL])
                    dps = pss.tile([128, 1024], F32, tag="s", name=f"psD{g}")
                    nc.tensor.matmul(dps[0:64, 0:COL], ematp[0:64, 0:64],
                                     t8[0:64, 0:COL], start=True, stop=True)
                    ln_ = sx.tile([128, BLKP], F32, tag=f"snl{g}")
                    nc.scalar.activation(ln_[0:64, 0:COL], dps[0:64, 0:COL],
                                         AF.Ln, bias=zeroc[0:64, :], scale=1.0)
                    rcp = sx.tile([128, BLKP], F32, tag=f"snr{g}")
                    nc.scalar.activation(rcp[0:64, 0:COL], ln_[0:64, 0:COL],
                                         AF.Exp, bias=zeroc[0:64, :], scale=-1.0)
                    a8 = sx.tile([128, BLKP], BF16, tag=f"sna{g}")
                    nc.vector.tensor_mul(a8[0:64, 0:COL], t8[0:64, 0:COL],
                                         rcp[0:64, 0:COL])
                    stageN[g] = a8

                psO = {}
                for gp in range(2):
                    g0, g1 = 2 * gp, 2 * gp + 1
                    psO[g0] = pso.tile([128, BLKP], F32, tag="acc",
                                       name=f"psO{g0}")
                    psO[g1] = pso.tile([128, BLKP], F32, tag="acc",
                                       name=f"psO{g1}")
                    for jt in range(NJT):
                        # 4 scores back-to-back (4 distinct PE positions),
                        # then the two PV pair-matmuls
                        psS0 = attn_scores(g0, jt)
                        psS1 = attn_scores(g1, jt)
                        attn_exp_pv(g0, jt, psS0)
                        attn_exp_pv(g1, jt, psS1)
                    stage_n(g0)
                    stage_n(g1)

                # ---- output projection ------------------------------------
                psP = pso.tile([128, BLKP], F32, tag="acc", name="psP")
                for g in range(4):
                    nc.tensor.matmul(psP[:, 0:COL], wpp_g(l, g)[0:64, :],
                                     stageN[g][0:64, 0:COL],
                                     start=(g == 0), stop=(g == 3))

                # ---- residual + LN1 ---------------------------------------
                y = sx.tile([128, BLKP], F32, tag="y")
                for (c0, c1) in ((0, 256), (256, COL)):
                    nc.vector.tensor_add(y[:, c0:c1], psP[:, c0:c1],
                                         xblk[:, c0:c1])
                layernorm(y, xblk, xblkb,
                          ln1gn[:, l:l + 1], ln1b[:, l:l + 1],
                          out_fp8=(xblk8 if l == 0 else None))

                # ---- AllGather of updated tokens between layers (fp8) -----
                if l == 0:
                    ag_in = dram.tile([128, BLK], FP8)
                    ag_out = dram.tile([NCORE * 128, BLK], FP8,
                                       addr_space="Shared")
                    nc.sync.dma_start(ag_in[:, 0:256], xblk8[:, 0:256])
                    nc.sync.dma_start(ag_in[:, 256:BLK], xblk8[:, 256:BLK])
                    nc.gpsimd.collective_compute(
                        "AllGather", OP.bypass,
                        replica_groups=[list(range(NCORE))],
                        ins=[ag_in.opt()], outs=[ag_out.opt()])
                    ag_v = ag_out.rearrange("(r c) n -> r c n", r=NCORE)
                    for rr in range(NCORE):
                        eng = nc.sync if rr % 2 == 0 else nc.scalar
                        eng.dma_start(
                            tokT2[:, BLK * rr:BLK * (rr + 1)], ag_v[rr])
                    nc.gpsimd.memset(tokT2[:, L:LP], 0.0)

            # ---- FFN + LN2 ------------------------------------------------
            for f in range(4):
                psH = pss.tile([128, 1024], F32, tag="s")
                nc.tensor.matmul(psH[:, 0:COL], w1s[:, 128 * f:128 * (f + 1)],
                                 xblkb[:, 0:COL], start=True, stop=True)
                nc.scalar.activation(g4[:, 512 * f:512 * f + COL],
                                     psH[:, 0:COL], AF.Gelu,
                                     bias=b1t[:, f:f + 1], scale=1.0)
            psF = pso.tile([128, BLKP], F32, tag="acc")
            for f in range(4):
                nc.tensor.matmul(psF[:, 0:COL], w2s[:, 128 * f:128 * (f + 1)],
                                 g4[:, 512 * f:512 * f + COL],
                                 start=(f == 0), stop=(f == 3))
            y2 = sx.tile([128, BLKP], F32, tag="y")
            nc.vector.scalar_tensor_tensor(out=y2[:, 0:COL], in0=psF[:, 0:COL],
                                           scalar=b2c[:], in1=xblk[:, 0:COL],
                                           op0=OP.add, op1=OP.add)
            final = sx.tile([128, BLKP], F32, tag="fin")
            finb = sx.tile([128, BLKP], BF16, tag="finb")
            layernorm(y2, final, finb, ln2gn[:], ln2b[:])
            nc.sync.dma_start(d_out[:, 0:256], final[:, 0:256])
            nc.scalar.dma_start(d_out[:, 256:BLK], final[:, 256:BLK])

    nc.compile()
    return nc


def _prep_inputs(z, r, seq_embed, Wq, Wk, Wv, Wp, beta, ln1_g, ln1_b,
                 ffn_w1, ffn_b1, ffn_w2, ffn_b2, ln2_g, ln2_b):
    """Host-side data layout prep (slicing, padding, small transposes)."""
    bf = ml_dtypes.bfloat16
    f8 = ml_dtypes.float8_e4m3
    f32 = np.float32
    z3f = np.asarray(z, f32).reshape(S, C, N)
    z3 = np.ascontiguousarray(z3f).astype(bf)
    seqeT = np.ascontiguousarray(np.asarray(seq_embed, f32).T)      # [C,S]

    def pad_heads(W):
        # [C, 64] -> [C, 128] with head q at cols 32q..32q+15, rest zero
        Wp_ = np.zeros((C, 128), f32)
        for q in range(4):
            Wp_[:, 32 * q:32 * q + 16] = W[:, 16 * q:16 * q + 16]
        return Wp_

    wk1 = np.zeros((2, C, C), f32)
    wk2 = np.zeros((2, C, C), f32)
    wqp = np.zeros((NL, 2, C, C), f32)
    for t in range(2):
        wk1[t] = pad_heads(np.asarray(Wk[0], f32)[:, 64 * t:64 * t + 64])
        wk2[t] = KV8 * pad_heads(np.asarray(Wk[1], f32)[:, 64 * t:64 * t + 64])
        for l in range(NL):
            wqp[l, t] = pad_heads(np.asarray(Wq[l], f32)[:, 64 * t:64 * t + 64])

    # V weights: psV col layout [h, 32] with dims at 0:16
    def v_pack(W, scale):
        wv = np.zeros((C, 256), f32)
        for h in range(H):
            wv[:, 32 * h:32 * h + 16] = scale * W[:, 16 * h:16 * h + 16]
        return wv
    wv1 = v_pack(np.asarray(Wv[0], f32), 1.0)
    wv2 = v_pack(np.asarray(Wv[1], f32), KV8)

    # output projection in pair-block layout: rows 0-15 = head 2g dims,
    # rows 32-47 = head 2g+1 dims; layer 2 divided by KV8
    wpp = np.zeros((NL, 4, 64, C), f32)
    for l in range(NL):
        sc = 1.0 / KV8 if l == 1 else 1.0
        for g in range(4):
            wpp[l, g, 0:16, :] = sc * np.asarray(Wp[l], f32)[
                16 * (2 * g):16 * (2 * g) + 16, :]
            wpp[l, g, 32:48, :] = sc * np.asarray(Wp[l], f32)[
                16 * (2 * g + 1):16 * (2 * g + 1) + 16, :]

    logr = np.log(np.asarray(r, np.float64)).astype(np.float64)     # [S]
    logr_tok = np.repeat(logr, N)                                   # [L]
    logrb = np.full((128, NL * NJT), ACT_PAD, f32)
    bvecb = np.full((128, NL * NJT), DVE_PAD, f32)
    for l in range(NL):
        bl = float(np.asarray(beta, f32)[l])
        cola = np.full(LP, ACT_PAD, np.float64)
        cola[:L] = bl * logr_tok
        colv = np.full(LP, DVE_PAD, np.float64)
        colv[:L] = 8.0 * (LOG2E * bl * logr_tok + 7.0 - SIGMA)
        logrb[:, l * NJT:(l + 1) * NJT] = cola.reshape(NJT, 128).T
        bvecb[:, l * NJT:(l + 1) * NJT] = colv.reshape(NJT, 128).T

    b1t = np.ascontiguousarray(np.asarray(ffn_b1, f32).reshape(4, C).T)
    b2c = np.asarray(ffn_b2, f32).reshape(C, 1)
    ln1gn = np.ascontiguousarray(-np.asarray(ln1_g, f32).T)         # [C,NL]
    ln1bT = np.ascontiguousarray(np.asarray(ln1_b, f32).T)
    ln2gn = (-np.asarray(ln2_g, f32)).reshape(C, 1)
    ln2bc = np.asarray(ln2_b, f32).reshape(C, 1)

    common = {
        "z3": z3,
        "seqeT": seqeT,
        "wk1": wk1.astype(bf), "wk2": wk2.astype(f8),
        "wqp": wqp.astype(bf),
        "wv1": wv1.astype(bf), "wv2": wv2.astype(f8),
        "wpp": wpp.astype(bf),
        "logrb": logrb, "bvecb": bvecb,
        "w1": np.asarray(ffn_w1, f32).astype(bf),
        "w2": np.asarray(ffn_w2, f32).astype(bf),
        "b1t": b1t, "b2c": b2c,
        "ln1gn": ln1gn, "ln1b": ln1bT,
        "ln2gn": ln2gn, "ln2b": ln2bc,
    }
    in_maps = []
    for k in range(NCORE):
        s = (k * BLK) // N
        off = (k * BLK) % N
        zblk = np.zeros((C, BLKP), f32)
        zblk[:, 0:BLK] = z3f[s, :, off:off + BLK]
        m = dict(common)
        m["zblk"] = zblk
        m["seqe"] = np.ascontiguousarray(seqeT[:, s:s + 1])
        in_maps.append(m)
    return in_maps


def kernel(z, r, seq_embed, Wq, Wk, Wv, Wp, beta, ln1_g, ln1_b,
           ffn_w1, ffn_b1, ffn_w2, ffn_b2, ln2_g, ln2_b, _results_out=None):
    from concourse.bass_utils import run_bass_kernel_spmd

    if "nc" not in _CACHE:
        _CACHE["nc"] = _build_nc()
    nc = _CACHE["nc"]

    in_maps = _prep_inputs(z, r, seq_embed, Wq, Wk, Wv, Wp, beta,
                           ln1_g, ln1_b, ffn_w1, ffn_b1, ffn_w2, ffn_b2,
                           ln2_g, ln2_b)
    res = run_bass_kernel_spmd(nc, in_maps, core_ids=list(range(NCORE)))
    if _results_out is not None:
        _results_out.append(res)

    blocks = [res.results[k]["outT"] for k in range(NCORE)]   # [C, BLK] each
    big = np.stack(blocks, axis=0).reshape(S, 2, C, BLK)
    tokTfin = big.transpose(0, 2, 1, 3).reshape(S, C, N)      # [s, c, n]
    out = tokTfin.reshape(1, S * C, 10, 10, 10).astype(np.float32)
    return out
